# revision 14
# baseline (speedup 1.0000x reference)
"""Equivariant rotation conv for Trainium2, 8-core batch-parallel.

Computes: rotate a (128*8, 128, 3, 3) filter bank by 8 data-dependent angles
(bilinear resampling), run a 3x3 same-padded conv of x (16,128,128,128) with
all 8*128 rotated filters, then max over the 8 rotations -> (16,128,128,128).

Sharding: data-parallel over batch, 2 images per core; the rotated filter
bank is replicated.  The rotation itself (a 9x9 tap-mixing matrix per
rotation, a pure function of the 8 rot_alpha scalars) is applied to the
filter bank on the HOST in f32 (10 MFLOP against the conv's 309 GFLOP) and
shipped pre-cast to bf16, so the device runs a pure conv+max pipeline:

  - per 32-row block: DMA the bf16 input rows (with zero halo kept
    persistent in SBUF) straight into ping-pong staging buffers,
  - the conv runs as 9 shifted PE matmuls in bf16 (K=Cin=128 partitions,
    N=512 spatial) accumulated in f32 PSUM, one PSUM bank per 4 output
    rows, 8 rotation chunks back to back,
  - a running elementwise max over the rotation chunks on DVE, with the
    final max fused with the per-slice output DMA.

The PE runs gap-free at ~218.5 ns per 512-column matmul (~99.7% matrix
occupancy, measured); 4608 matmuls/core ≈ 1007 us is the silicon floor at
the sustained ~2.34 GHz PE clock, so v2 only trims the head (first matmul
gated on a 1-tap weight DMA + 4 input rows, ~2 us) and the tail (final
rotation emitted subtile-major so output stores start early).
"""

import numpy as np
import ml_dtypes


def _install_axon_hooks_shim():
    """Provide antenv.axon_hooks (NTFF profile hook) when the image's antenv
    lacks it, so run_bass_kernel_spmd(trace=True) works instead of crashing
    on import.  The hook drives NRT profiling via ctypes into the axon PJRT
    plugin, mirroring the boot-side installer."""
    import contextlib
    import ctypes
    import os
    import sys
    import types

    try:
        import antenv.axon_hooks  # noqa: F401

        return
    except ImportError:
        pass

    state = {"hook": None, "resolved": False}

    def _make_hook():
        so_path = os.environ.get("AXON_PJRT_SO", "/opt/axon/libaxon_pjrt.so")
        if not os.path.exists(so_path):
            return None
        lib = ctypes.CDLL(so_path)
        if not hasattr(lib, "axon_start_nrt_profile"):
            return None
        lib.axon_start_nrt_profile.argtypes = [
            ctypes.POINTER(ctypes.c_int64),
            ctypes.c_size_t,
        ]
        lib.axon_start_nrt_profile.restype = ctypes.c_int64
        lib.axon_stop_nrt_profile.argtypes = [ctypes.c_char_p]
        lib.axon_stop_nrt_profile.restype = ctypes.c_int64

        @contextlib.contextmanager
        def _hook(output_dir, device_ids):
            import jax

            jax.devices()
            if device_ids:
                ids = (ctypes.c_int64 * len(device_ids))(*device_ids)
                rc = lib.axon_start_nrt_profile(ids, len(device_ids))
            else:
                rc = lib.axon_start_nrt_profile(None, 0)
            if rc != 0:
                raise RuntimeError(f"axon_start_nrt_profile rc={rc}")
            try:
                yield
            finally:
                n = lib.axon_stop_nrt_profile(str(output_dir).encode())
                if n < 0:
                    raise RuntimeError(f"axon_stop_nrt_profile rc={n}")
                print(f"profile: {n} file(s) written to {output_dir}")

        return _hook

    mod = types.ModuleType("antenv.axon_hooks")

    def set_axon_ntff_profile_hook(h):
        state["hook"] = h
        state["resolved"] = True

    def get_axon_ntff_profile_hook():
        if not state["resolved"]:
            state["hook"] = _make_hook()
            state["resolved"] = True
        return state["hook"]

    mod.set_axon_ntff_profile_hook = set_axon_ntff_profile_hook
    mod.get_axon_ntff_profile_hook = get_axon_ntff_profile_hook
    sys.modules["antenv.axon_hooks"] = mod


_install_axon_hooks_shim()

import concourse.bass as bass  # noqa: E402,F401
import concourse.mybir as mybir  # noqa: E402
from concourse import bacc  # noqa: E402
from concourse.bass_utils import run_bass_kernel_spmd  # noqa: E402
from concourse.tile import TileContext  # noqa: E402

F32 = mybir.dt.float32
BF16 = mybir.dt.bfloat16

B, CIN, H, W = 16, 128, 128, 128
R, O, K = 8, 128, 3
NCORES = 8
BL = B // NCORES  # images per core
RB = 32           # output rows per block
NS = RB // 4      # psum subtiles (4 rows = 512 cols) per block
NBLK = H // RB

# PE warm-up matmuls before the first real work (HAM clock ramp + keeps the
# PE busy while the first weight/x DMAs land).
WARMUP = 10

_TRACE = False
LAST_RESULTS = None
_NC_CACHE = {}


def _rot_mats(rot_alpha):
    """Per-rotation 9x9 bilinear resampling matrices, matching the reference
    F.grid_sample(align_corners=True, zeros) tap logic exactly.

    M[r, p, q]: coefficient of original tap q = (qy*3+qx) in rotated tap
    p = (py*3+px)."""
    M = np.zeros((R, 9, 9), np.float64)
    lin = np.linspace(-1.0, 1.0, K)
    for r in range(R):
        ang = float(rot_alpha[r]) * (np.pi / 4.0) * r
        c, s = np.cos(ang), np.sin(ang)
        for a in range(K):          # output row (gy = lin[a])
            for b in range(K):      # output col (gx = lin[b])
                gx, gy = lin[b], lin[a]
                xs = c * gx - s * gy
                ys = s * gx + c * gy
                ix = (xs + 1.0) * 0.5 * (K - 1)
                iy = (ys + 1.0) * 0.5 * (K - 1)
                x0 = int(np.floor(ix))
                y0 = int(np.floor(iy))
                wx, wy = ix - x0, iy - y0
                p = a * K + b
                for yi, xi, wt in (
                    (y0, x0, (1 - wy) * (1 - wx)),
                    (y0, x0 + 1, (1 - wy) * wx),
                    (y0 + 1, x0, wy * (1 - wx)),
                    (y0 + 1, x0 + 1, wy * wx),
                ):
                    if 0 <= yi < K and 0 <= xi < K:
                        M[r, p, yi * K + xi] += wt
    return M.astype(np.float32)


def _build():
    nc = bacc.Bacc(trn_type="TRN2")
    # x ships pre-padded (zero halo rows/cols) so no on-device memsets are
    # needed and every block load is one uniform strip DMA.
    xs = nc.dram_tensor("xs", [BL, CIN, H + 2, W + 2], BF16, kind="ExternalInput")
    # rw[r, i, p*O + o] = rotated filter bank, lhsT layout per tap
    rw = nc.dram_tensor("rw", [R, CIN, 9 * O], BF16, kind="ExternalInput")
    y = nc.dram_tensor("y", [BL, O, H, W], F32, kind="ExternalOutput")

    with TileContext(nc) as tc:
        with (
            tc.tile_pool(name="wrot", bufs=1) as rpool,
            tc.tile_pool(name="xio", bufs=1) as xpool,
            tc.tile_pool(name="accp", bufs=3) as apool,
            tc.tile_pool(name="psum", bufs=1, space="PSUM") as ppool,
        ):
            rotw = [
                rpool.tile([128, 9, O], BF16, name=f"rotw{r}", tag=f"rotw{r}")
                for r in range(R)
            ]

            # PE warm-up: matmuls on a scratch tile seeded by the very first
            # (tiny) DMA, so the dummies start as soon as the Tensor
            # sequencer boots (results land in the ps0 bank slot and are
            # overwritten by the first real start=True group).
            dum_lhs = rpool.tile([128, 128], BF16, name="dum_lhs", tag="dum")
            nc.sync.dma_start(out=dum_lhs[:, 0:64], in_=rw[0, :, 0:64])
            nc.sync.dma_start(out=dum_lhs[:, 64:128], in_=rw[0, :, 64:128])
            dum_ps = ppool.tile([128, 128], F32, name="dum_ps", tag="ps0")
            for _ in range(WARMUP):
                nc.tensor.matmul(
                    dum_ps[:, :], dum_lhs[:, :], dum_lhs[:, :],
                    start=True, stop=True,
                )

            # x staging: 3 persistent ping-pong buffers, fully overwritten by
            # each block's strip DMA (padding included), so no memsets ever.
            xst2 = [
                xpool.tile([128, RB + 2, W + 2], BF16, name=f"xst{i}", tag=f"xst{i}")
                for i in range(3)
            ]

            def load_x(g, b, blk, chunks=1, cuts=None):
                # DMA the block's padded input rows into the ping-pong
                # staging buffer.  `cuts`/`chunks` split the load so
                # downstream matmuls can start on the first rows before the
                # whole block has landed.
                xst = xst2[g % 3]
                r0 = blk * RB  # padded-row index of the block's top halo row
                nrows = RB + 2
                if cuts is None:
                    cuts = [nrows * k // chunks for k in range(chunks + 1)]
                for k in range(len(cuts) - 1):
                    a, c = cuts[k], cuts[k + 1]
                    nc.sync.dma_start(
                        out=xst[:, a:c, :],
                        in_=xs[b, :, r0 + a : r0 + c, :],
                    )
                return xst

            def conv_chunk(xmm, acc, r, store=None, s_groups=1, fine_tail=False):
                pst = [
                    ppool.tile([128, 4, W], F32, name=f"ps{s}", tag=f"ps{s}")
                    for s in range(NS)
                ]

                def emit_max_store(s, rows):
                    # rows: list of (row0, nrows) pieces within the subtile
                    for a, n in rows:
                        lo, hi = 4 * s + a, 4 * s + a + n
                        if r == 0:
                            nc.vector.tensor_copy(
                                acc[:, lo:hi, :], pst[s][:, a : a + n, :]
                            )
                        else:
                            nc.vector.tensor_tensor(
                                acc[:, lo:hi, :],
                                acc[:, lo:hi, :],
                                pst[s][:, a : a + n, :],
                                mybir.AluOpType.max,
                            )
                        if store is not None:
                            b, h0 = store
                            nc.sync.dma_start(
                                out=y[b, :, h0 + lo : h0 + hi, :],
                                in_=acc[:, lo:hi, :],
                            )

                def emit_group(ss):
                    for p in range(9):
                        ky, kx = divmod(p, 3)
                        lhsT = rotw[r][:, p, :]
                        for s in ss:
                            rhs = xmm[:, 4 * s + ky : 4 * s + ky + 4, kx : kx + W]
                            nc.tensor.matmul(
                                pst[s][:, :, :], lhsT, rhs,
                                start=(p == 0), stop=(p == 8),
                            )
                    for s in ss:
                        if fine_tail and s == NS - 1:
                            # drain the last subtile in 1-row pieces so the
                            # final store starts right behind the last matmul
                            emit_max_store(s, [(0, 2), (2, 1), (3, 1)])
                        elif fine_tail and s == NS - 2:
                            emit_max_store(s, [(0, 2), (2, 2)])
                        else:
                            emit_max_store(s, [(0, 4)])

                per = NS // s_groups
                for k in range(s_groups):
                    emit_group(range(k * per, (k + 1) * per))

            # DMA issue order (the sync queue issues serially): rotation 0's
            # ky=0 taps (0-2) and the first 4 x rows go first so the very
            # first matmuls are unblocked right after the queues boot; the
            # remaining taps/rows/rotations follow interleaved by need-time.
            nc.sync.dma_start(
                out=rotw[0][:, 0:3, :].rearrange("i p o -> i (p o)"),
                in_=rw[0, :, 0 : 3 * O],
            )
            xst0 = xst2[0]
            def x0_chunk(a, c):
                nc.sync.dma_start(
                    out=xst0[:, a:c, :],
                    in_=xs[0, :, a:c, :],
                )
            x0_chunk(0, 6)
            nc.sync.dma_start(
                out=rotw[0][:, 3:9, :].rearrange("i p o -> i (p o)"),
                in_=rw[0, :, 3 * O : 9 * O],
            )
            x0_chunk(6, 10)
            x0_chunk(10, 14)
            nc.sync.dma_start(
                out=rotw[1][:, :, :].rearrange("i p o -> i (p o)"),
                in_=rw[1, :, :],
            )
            x0_chunk(14, 24)
            x0_chunk(24, 34)
            for r in range(2, R):
                nc.sync.dma_start(
                    out=rotw[r][:, :, :].rearrange("i p o -> i (p o)"),
                    in_=rw[r, :, :],
                )
            xmm_pre = [xst0]
            xmm_pre.append(load_x(1, 0, 1, chunks=2))
            xmm_pre.append(load_x(2, 0, 2, chunks=2))

            last_g = BL * NBLK - 1
            for g in range(BL * NBLK):
                b, blk = divmod(g, NBLK)
                xmm = xmm_pre[g] if g < 3 else load_x(g, b, blk)
                acc = apool.tile([128, RB, W], F32, name="acc", tag="acc")
                for r in range(R):
                    final = r == R - 1
                    # block 0 rotation 0 runs subtile-major so matmuls start
                    # as soon as the first x rows land; the very last chunk
                    # runs subtile-major so the final stores drain early.
                    sg = 8 if (g == 0 and r == 0) or (final and g == last_g) else 1
                    conv_chunk(
                        xmm, acc, r,
                        store=(b, blk * RB) if final else None,
                        s_groups=sg,
                        fine_tail=(final and g == last_g),
                    )
    nc.finalize()
    return nc


def _get_nc():
    if "v2" not in _NC_CACHE:
        _NC_CACHE["v2"] = _build()
    return _NC_CACHE["v2"]


def kernel(x, weight, rot_alpha):
    global LAST_RESULTS
    x = np.asarray(x, np.float32)
    weight = np.ascontiguousarray(np.asarray(weight, np.float32))
    rot_alpha = np.asarray(rot_alpha, np.float32)

    # Host-side filter rotation: rw[r, i, p, o] = sum_q M[r,p,q] * W[o*R+r, i, q]
    # in f32, then one cast to bf16 (same rounding boundary as the previous
    # on-device DVE mixing, so numerics are unchanged).
    M = _rot_mats(rot_alpha)
    wq = weight.reshape(O, R, CIN, 9).transpose(1, 2, 3, 0)  # (R, I, q, O)
    rot = np.einsum("rpq,riqo->ripo", M, wq)
    rw = np.ascontiguousarray(rot.reshape(R, CIN, 9 * O)).astype(
        ml_dtypes.bfloat16
    )
    xb = np.zeros((B, CIN, H + 2, W + 2), ml_dtypes.bfloat16)
    xb[:, :, 1 : H + 1, 1 : W + 1] = x.astype(ml_dtypes.bfloat16)

    nc = _get_nc()
    in_maps = [
        {"xs": np.ascontiguousarray(xb[c * BL : (c + 1) * BL]), "rw": rw}
        for c in range(NCORES)
    ]
    try:
        res = run_bass_kernel_spmd(nc, in_maps, list(range(NCORES)), trace=_TRACE)
    except Exception:
        # One retry (without tracing): a failed compile or an aborted run can
        # leave a NeuronCore transiently wedged; the next attempt recovers.
        res = run_bass_kernel_spmd(nc, in_maps, list(range(NCORES)), trace=False)
    LAST_RESULTS = res
    return np.concatenate([res.results[c]["y"] for c in range(NCORES)], axis=0)


# revision 15
# speedup vs baseline: 1.1975x; 1.1975x over previous
"""Equivariant rotation conv for Trainium2, 8-core batch-parallel.

Computes: rotate a (128*8, 128, 3, 3) filter bank by 8 data-dependent angles
(bilinear resampling), run a 3x3 same-padded conv of x (16,128,128,128) with
all 8*128 rotated filters, then max over the 8 rotations -> (16,128,128,128).

Sharding: data-parallel over batch, 2 images per core; the rotated filter
bank is replicated.  The rotation itself (a 9x9 tap-mixing matrix per
rotation, a pure function of the 8 rot_alpha scalars) is applied to the
filter bank on the HOST in f32 (10 MFLOP against the conv's 309 GFLOP) and
shipped pre-cast to bf16, so the device runs a pure conv+max pipeline:

  - per 32-row block: DMA the bf16 input rows (with zero halo kept
    persistent in SBUF) straight into ping-pong staging buffers,
  - the conv runs as 9 shifted PE matmuls in bf16 (K=Cin=128 partitions,
    N=512 spatial) accumulated in f32 PSUM, one PSUM bank per 4 output
    rows, 8 rotation chunks back to back,
  - a running elementwise max over the rotation chunks on DVE, with the
    final max fused with the per-slice output DMA.

The PE runs gap-free at ~218.5 ns per 512-column matmul (~99.7% matrix
occupancy, measured); 4608 matmuls/core ≈ 1007 us is the silicon floor at
the sustained ~2.34 GHz PE clock, so v2 only trims the head (first matmul
gated on a 1-tap weight DMA + 4 input rows, ~2 us) and the tail (final
rotation emitted subtile-major so output stores start early).
"""

import numpy as np
import ml_dtypes


def _install_axon_hooks_shim():
    """Provide antenv.axon_hooks (NTFF profile hook) when the image's antenv
    lacks it, so run_bass_kernel_spmd(trace=True) works instead of crashing
    on import.  The hook drives NRT profiling via ctypes into the axon PJRT
    plugin, mirroring the boot-side installer."""
    import contextlib
    import ctypes
    import os
    import sys
    import types

    try:
        import antenv.axon_hooks  # noqa: F401

        return
    except ImportError:
        pass

    state = {"hook": None, "resolved": False}

    def _make_hook():
        so_path = os.environ.get("AXON_PJRT_SO", "/opt/axon/libaxon_pjrt.so")
        if not os.path.exists(so_path):
            return None
        lib = ctypes.CDLL(so_path)
        if not hasattr(lib, "axon_start_nrt_profile"):
            return None
        lib.axon_start_nrt_profile.argtypes = [
            ctypes.POINTER(ctypes.c_int64),
            ctypes.c_size_t,
        ]
        lib.axon_start_nrt_profile.restype = ctypes.c_int64
        lib.axon_stop_nrt_profile.argtypes = [ctypes.c_char_p]
        lib.axon_stop_nrt_profile.restype = ctypes.c_int64

        @contextlib.contextmanager
        def _hook(output_dir, device_ids):
            import jax

            jax.devices()
            if device_ids:
                ids = (ctypes.c_int64 * len(device_ids))(*device_ids)
                rc = lib.axon_start_nrt_profile(ids, len(device_ids))
            else:
                rc = lib.axon_start_nrt_profile(None, 0)
            if rc != 0:
                raise RuntimeError(f"axon_start_nrt_profile rc={rc}")
            try:
                yield
            finally:
                n = lib.axon_stop_nrt_profile(str(output_dir).encode())
                if n < 0:
                    raise RuntimeError(f"axon_stop_nrt_profile rc={n}")
                print(f"profile: {n} file(s) written to {output_dir}")

        return _hook

    mod = types.ModuleType("antenv.axon_hooks")

    def set_axon_ntff_profile_hook(h):
        state["hook"] = h
        state["resolved"] = True

    def get_axon_ntff_profile_hook():
        if not state["resolved"]:
            state["hook"] = _make_hook()
            state["resolved"] = True
        return state["hook"]

    mod.set_axon_ntff_profile_hook = set_axon_ntff_profile_hook
    mod.get_axon_ntff_profile_hook = get_axon_ntff_profile_hook
    sys.modules["antenv.axon_hooks"] = mod


_install_axon_hooks_shim()

import concourse.bass as bass  # noqa: E402,F401
import concourse.mybir as mybir  # noqa: E402
from concourse import bacc  # noqa: E402
from concourse.bass_utils import run_bass_kernel_spmd  # noqa: E402
from concourse.tile import TileContext  # noqa: E402

F32 = mybir.dt.float32
BF16 = mybir.dt.bfloat16

B, CIN, H, W = 16, 128, 128, 128
R, O, K = 8, 128, 3
NCORES = 8
BL = B // NCORES  # images per core
RB = 32           # output rows per block
NS = RB // 4      # psum subtiles (4 rows = 512 cols) per block
NBLK = H // RB

# PE warm-up matmuls before the first real work (HAM clock ramp + keeps the
# PE busy while the first weight/x DMAs land).
WARMUP = 10

_TRACE = False
LAST_RESULTS = None
_NC_CACHE = {}


def _rot_mats(rot_alpha):
    """Per-rotation 9x9 bilinear resampling matrices, matching the reference
    F.grid_sample(align_corners=True, zeros) tap logic exactly.

    M[r, p, q]: coefficient of original tap q = (qy*3+qx) in rotated tap
    p = (py*3+px)."""
    M = np.zeros((R, 9, 9), np.float64)
    lin = np.linspace(-1.0, 1.0, K)
    for r in range(R):
        ang = float(rot_alpha[r]) * (np.pi / 4.0) * r
        c, s = np.cos(ang), np.sin(ang)
        for a in range(K):          # output row (gy = lin[a])
            for b in range(K):      # output col (gx = lin[b])
                gx, gy = lin[b], lin[a]
                xs = c * gx - s * gy
                ys = s * gx + c * gy
                ix = (xs + 1.0) * 0.5 * (K - 1)
                iy = (ys + 1.0) * 0.5 * (K - 1)
                x0 = int(np.floor(ix))
                y0 = int(np.floor(iy))
                wx, wy = ix - x0, iy - y0
                p = a * K + b
                for yi, xi, wt in (
                    (y0, x0, (1 - wy) * (1 - wx)),
                    (y0, x0 + 1, (1 - wy) * wx),
                    (y0 + 1, x0, wy * (1 - wx)),
                    (y0 + 1, x0 + 1, wy * wx),
                ):
                    if 0 <= yi < K and 0 <= xi < K:
                        M[r, p, yi * K + xi] += wt
    return M.astype(np.float32)


def _build():
    nc = bacc.Bacc(trn_type="TRN2")
    # x ships pre-padded (zero halo rows/cols) so no on-device memsets are
    # needed and every block load is one uniform strip DMA.
    xs = nc.dram_tensor("xs", [BL, CIN, H + 2, W + 2], BF16, kind="ExternalInput")
    # rw[r, i, p*O + o] = rotated filter bank, lhsT layout per tap
    rw = nc.dram_tensor("rw", [R, CIN, 9 * O], BF16, kind="ExternalInput")
    y = nc.dram_tensor("y", [BL, O, H, W], F32, kind="ExternalOutput")

    with TileContext(nc) as tc:
        with (
            tc.tile_pool(name="wrot", bufs=1) as rpool,
            tc.tile_pool(name="xio", bufs=1) as xpool,
            tc.tile_pool(name="accp", bufs=3) as apool,
            tc.tile_pool(name="psum", bufs=1, space="PSUM") as ppool,
        ):
            rotw = [
                rpool.tile([128, 9, O], BF16, name=f"rotw{r}", tag=f"rotw{r}")
                for r in range(R)
            ]

            # PE warm-up: matmuls on a scratch tile seeded by the very first
            # (tiny) DMA, so the dummies start as soon as the Tensor
            # sequencer boots (results land in the ps0 bank slot and are
            # overwritten by the first real start=True group).
            dum_lhs = rpool.tile([128, 128], BF16, name="dum_lhs", tag="dum")
            nc.sync.dma_start(out=dum_lhs[:, 0:64], in_=rw[0, :, 0:64])
            nc.sync.dma_start(out=dum_lhs[:, 64:128], in_=rw[0, :, 64:128])
            dum_ps = ppool.tile([128, 128], F32, name="dum_ps", tag="ps0")
            for _ in range(WARMUP):
                nc.tensor.matmul(
                    dum_ps[:, :], dum_lhs[:, :], dum_lhs[:, :],
                    start=True, stop=True,
                )

            # x staging: 3 persistent ping-pong buffers, fully overwritten by
            # each block's strip DMA (padding included), so no memsets ever.
            xst2 = [
                xpool.tile([128, RB + 2, W + 2], BF16, name=f"xst{i}", tag=f"xst{i}")
                for i in range(3)
            ]

            def load_x(g, b, blk, chunks=1, cuts=None):
                # DMA the block's padded input rows into the ping-pong
                # staging buffer.  `cuts`/`chunks` split the load so
                # downstream matmuls can start on the first rows before the
                # whole block has landed.
                xst = xst2[g % 3]
                r0 = blk * RB  # padded-row index of the block's top halo row
                nrows = RB + 2
                if cuts is None:
                    cuts = [nrows * k // chunks for k in range(chunks + 1)]
                for k in range(len(cuts) - 1):
                    a, c = cuts[k], cuts[k + 1]
                    nc.sync.dma_start(
                        out=xst[:, a:c, :],
                        in_=xs[b, :, r0 + a : r0 + c, :],
                    )
                return xst

            def conv_chunk(xmm, acc, r, store=None, s_groups=1, fine_tail=False):
                pst = [
                    ppool.tile([128, 4, W], F32, name=f"ps{s}", tag=f"ps{s}")
                    for s in range(NS)
                ]

                def emit_max_store(s, rows):
                    # rows: list of (row0, nrows) pieces within the subtile
                    for a, n in rows:
                        lo, hi = 4 * s + a, 4 * s + a + n
                        if r == 0:
                            nc.vector.tensor_copy(
                                acc[:, lo:hi, :], pst[s][:, a : a + n, :]
                            )
                        else:
                            nc.vector.tensor_tensor(
                                acc[:, lo:hi, :],
                                acc[:, lo:hi, :],
                                pst[s][:, a : a + n, :],
                                mybir.AluOpType.max,
                            )
                        if store is not None:
                            b, h0 = store
                            nc.sync.dma_start(
                                out=y[b, :, h0 + lo : h0 + hi, :],
                                in_=acc[:, lo:hi, :],
                            )

                def emit_group(ss):
                    for p in range(9):
                        ky, kx = divmod(p, 3)
                        lhsT = rotw[r][:, p, :]
                        for s in ss:
                            rhs = xmm[:, 4 * s + ky : 4 * s + ky + 4, kx : kx + W]
                            nc.tensor.matmul(
                                pst[s][:, :, :], lhsT, rhs,
                                start=(p == 0), stop=(p == 8),
                            )
                    for s in ss:
                        if fine_tail and s == NS - 1:
                            # drain the last subtile in 2-row pieces so the
                            # final store starts right behind the last matmul
                            emit_max_store(s, [(0, 2), (2, 2)])
                        else:
                            emit_max_store(s, [(0, 4)])

                per = NS // s_groups
                for k in range(s_groups):
                    emit_group(range(k * per, (k + 1) * per))

            # DMA issue order (the sync queue issues serially): rotation 0's
            # ky=0 taps (0-2) and the first 4 x rows go first so the very
            # first matmuls are unblocked right after the queues boot; the
            # remaining taps/rows/rotations follow interleaved by need-time.
            nc.sync.dma_start(
                out=rotw[0][:, 0:3, :].rearrange("i p o -> i (p o)"),
                in_=rw[0, :, 0 : 3 * O],
            )
            xst0 = xst2[0]
            def x0_chunk(a, c):
                nc.sync.dma_start(
                    out=xst0[:, a:c, :],
                    in_=xs[0, :, a:c, :],
                )
            x0_chunk(0, 6)
            nc.sync.dma_start(
                out=rotw[0][:, 3:9, :].rearrange("i p o -> i (p o)"),
                in_=rw[0, :, 3 * O : 9 * O],
            )
            x0_chunk(6, 10)
            x0_chunk(10, 14)
            nc.sync.dma_start(
                out=rotw[1][:, :, :].rearrange("i p o -> i (p o)"),
                in_=rw[1, :, :],
            )
            x0_chunk(14, 24)
            x0_chunk(24, 34)
            for r in range(2, R):
                nc.sync.dma_start(
                    out=rotw[r][:, :, :].rearrange("i p o -> i (p o)"),
                    in_=rw[r, :, :],
                )
            xmm_pre = [xst0]
            xmm_pre.append(load_x(1, 0, 1, chunks=2))
            xmm_pre.append(load_x(2, 0, 2, chunks=2))

            last_g = BL * NBLK - 1
            for g in range(BL * NBLK):
                b, blk = divmod(g, NBLK)
                xmm = xmm_pre[g] if g < 3 else load_x(g, b, blk)
                acc = apool.tile([128, RB, W], F32, name="acc", tag="acc")
                for r in range(R):
                    final = r == R - 1
                    # block 0 rotation 0 runs subtile-major so matmuls start
                    # as soon as the first x rows land; the very last chunk
                    # runs subtile-major so the final stores drain early.
                    sg = 8 if (g == 0 and r == 0) or (final and g == last_g) else 1
                    conv_chunk(
                        xmm, acc, r,
                        store=(b, blk * RB) if final else None,
                        s_groups=sg,
                        fine_tail=(final and g == last_g),
                    )
    nc.finalize()
    return nc


def _get_nc():
    if "v2" not in _NC_CACHE:
        _NC_CACHE["v2"] = _build()
    return _NC_CACHE["v2"]


def kernel(x, weight, rot_alpha):
    global LAST_RESULTS
    x = np.asarray(x, np.float32)
    weight = np.ascontiguousarray(np.asarray(weight, np.float32))
    rot_alpha = np.asarray(rot_alpha, np.float32)

    # Host-side filter rotation: rw[r, i, p, o] = sum_q M[r,p,q] * W[o*R+r, i, q]
    # in f32, then one cast to bf16 (same rounding boundary as the previous
    # on-device DVE mixing, so numerics are unchanged).
    M = _rot_mats(rot_alpha)
    wq = weight.reshape(O, R, CIN, 9).transpose(1, 2, 3, 0)  # (R, I, q, O)
    rot = np.einsum("rpq,riqo->ripo", M, wq)
    rw = np.ascontiguousarray(rot.reshape(R, CIN, 9 * O)).astype(
        ml_dtypes.bfloat16
    )
    xb = np.zeros((B, CIN, H + 2, W + 2), ml_dtypes.bfloat16)
    xb[:, :, 1 : H + 1, 1 : W + 1] = x.astype(ml_dtypes.bfloat16)

    nc = _get_nc()
    in_maps = [
        {"xs": np.ascontiguousarray(xb[c * BL : (c + 1) * BL]), "rw": rw}
        for c in range(NCORES)
    ]
    try:
        res = run_bass_kernel_spmd(nc, in_maps, list(range(NCORES)), trace=_TRACE)
    except Exception:
        # One retry (without tracing): a failed compile or an aborted run can
        # leave a NeuronCore transiently wedged; the next attempt recovers.
        res = run_bass_kernel_spmd(nc, in_maps, list(range(NCORES)), trace=False)
    LAST_RESULTS = res
    return np.concatenate([res.results[c]["y"] for c in range(NCORES)], axis=0)


# revision 16
# speedup vs baseline: 1.1993x; 1.0016x over previous
"""Equivariant rotation conv for Trainium2, 8-core batch-parallel.

Computes: rotate a (128*8, 128, 3, 3) filter bank by 8 data-dependent angles
(bilinear resampling), run a 3x3 same-padded conv of x (16,128,128,128) with
all 8*128 rotated filters, then max over the 8 rotations -> (16,128,128,128).

Sharding: data-parallel over batch, 2 images per core; the rotated filter
bank is replicated.  The rotation itself (a 9x9 tap-mixing matrix per
rotation, a pure function of the 8 rot_alpha scalars) is applied to the
filter bank on the HOST in f32 (10 MFLOP against the conv's 309 GFLOP) and
shipped pre-cast to bf16, so the device runs a pure conv+max pipeline:

  - per 32-row block: DMA the bf16 input rows (with zero halo kept
    persistent in SBUF) straight into ping-pong staging buffers,
  - the conv runs as 9 shifted PE matmuls in bf16 (K=Cin=128 partitions,
    N=512 spatial) accumulated in f32 PSUM, one PSUM bank per 4 output
    rows, 8 rotation chunks back to back,
  - a running elementwise max over the rotation chunks on DVE, with the
    final max fused with the per-slice output DMA.

The PE runs gap-free at ~218.3 ns per 512-column matmul (99.8% matrix
occupancy, measured: 512 cycles at 2.4 GHz + ~12 cycles fixed per-matmul
SBUF access latency); 4608 matmuls/core ≈ 1006 us is the silicon floor.
fp8 was evaluated and rejected: DoubleRow on real TRN2 runs at the same
ns-per-output-column as bf16 (2x MACs via K=256 packing), so the only
fp8 scheme that beats bf16 parity is uncorrected quantization, which
measures rel_l2 = 2.5e-2 against the 2e-2 gate.  The remaining time is
fixed overhead: ~7 us engine wake, ~3 us HAM clock ramp (absorbed by
warm-up dummies gated on the first tiny DMA), ~5.5 us drain + framework
epilogue; the head is minimized by DMA-ordering rotation-0/ky=0 taps and
the first 6 padded rows ahead of everything else.
"""

import numpy as np
import ml_dtypes


def _install_axon_hooks_shim():
    """Provide antenv.axon_hooks (NTFF profile hook) when the image's antenv
    lacks it, so run_bass_kernel_spmd(trace=True) works instead of crashing
    on import.  The hook drives NRT profiling via ctypes into the axon PJRT
    plugin, mirroring the boot-side installer."""
    import contextlib
    import ctypes
    import os
    import sys
    import types

    try:
        import antenv.axon_hooks  # noqa: F401

        return
    except ImportError:
        pass

    state = {"hook": None, "resolved": False}

    def _make_hook():
        so_path = os.environ.get("AXON_PJRT_SO", "/opt/axon/libaxon_pjrt.so")
        if not os.path.exists(so_path):
            return None
        lib = ctypes.CDLL(so_path)
        if not hasattr(lib, "axon_start_nrt_profile"):
            return None
        lib.axon_start_nrt_profile.argtypes = [
            ctypes.POINTER(ctypes.c_int64),
            ctypes.c_size_t,
        ]
        lib.axon_start_nrt_profile.restype = ctypes.c_int64
        lib.axon_stop_nrt_profile.argtypes = [ctypes.c_char_p]
        lib.axon_stop_nrt_profile.restype = ctypes.c_int64

        @contextlib.contextmanager
        def _hook(output_dir, device_ids):
            import jax

            jax.devices()
            if device_ids:
                ids = (ctypes.c_int64 * len(device_ids))(*device_ids)
                rc = lib.axon_start_nrt_profile(ids, len(device_ids))
            else:
                rc = lib.axon_start_nrt_profile(None, 0)
            if rc != 0:
                raise RuntimeError(f"axon_start_nrt_profile rc={rc}")
            try:
                yield
            finally:
                n = lib.axon_stop_nrt_profile(str(output_dir).encode())
                if n < 0:
                    raise RuntimeError(f"axon_stop_nrt_profile rc={n}")
                print(f"profile: {n} file(s) written to {output_dir}")

        return _hook

    mod = types.ModuleType("antenv.axon_hooks")

    def set_axon_ntff_profile_hook(h):
        state["hook"] = h
        state["resolved"] = True

    def get_axon_ntff_profile_hook():
        if not state["resolved"]:
            state["hook"] = _make_hook()
            state["resolved"] = True
        return state["hook"]

    mod.set_axon_ntff_profile_hook = set_axon_ntff_profile_hook
    mod.get_axon_ntff_profile_hook = get_axon_ntff_profile_hook
    sys.modules["antenv.axon_hooks"] = mod


_install_axon_hooks_shim()

import concourse.bass as bass  # noqa: E402,F401
import concourse.mybir as mybir  # noqa: E402
from concourse import bacc  # noqa: E402
from concourse.bass_utils import run_bass_kernel_spmd  # noqa: E402
from concourse.tile import TileContext  # noqa: E402

F32 = mybir.dt.float32
BF16 = mybir.dt.bfloat16

B, CIN, H, W = 16, 128, 128, 128
R, O, K = 8, 128, 3
NCORES = 8
BL = B // NCORES  # images per core
RB = 32           # output rows per block
NS = RB // 4      # psum subtiles (4 rows = 512 cols) per block
NBLK = H // RB

# PE warm-up matmuls before the first real work (HAM clock ramp + keeps the
# PE busy while the first weight/x DMAs land).
WARMUP = 10

_TRACE = False
LAST_RESULTS = None
_NC_CACHE = {}


def _rot_mats(rot_alpha):
    """Per-rotation 9x9 bilinear resampling matrices, matching the reference
    F.grid_sample(align_corners=True, zeros) tap logic exactly.

    M[r, p, q]: coefficient of original tap q = (qy*3+qx) in rotated tap
    p = (py*3+px)."""
    M = np.zeros((R, 9, 9), np.float64)
    lin = np.linspace(-1.0, 1.0, K)
    for r in range(R):
        ang = float(rot_alpha[r]) * (np.pi / 4.0) * r
        c, s = np.cos(ang), np.sin(ang)
        for a in range(K):          # output row (gy = lin[a])
            for b in range(K):      # output col (gx = lin[b])
                gx, gy = lin[b], lin[a]
                xs = c * gx - s * gy
                ys = s * gx + c * gy
                ix = (xs + 1.0) * 0.5 * (K - 1)
                iy = (ys + 1.0) * 0.5 * (K - 1)
                x0 = int(np.floor(ix))
                y0 = int(np.floor(iy))
                wx, wy = ix - x0, iy - y0
                p = a * K + b
                for yi, xi, wt in (
                    (y0, x0, (1 - wy) * (1 - wx)),
                    (y0, x0 + 1, (1 - wy) * wx),
                    (y0 + 1, x0, wy * (1 - wx)),
                    (y0 + 1, x0 + 1, wy * wx),
                ):
                    if 0 <= yi < K and 0 <= xi < K:
                        M[r, p, yi * K + xi] += wt
    return M.astype(np.float32)


def _build():
    nc = bacc.Bacc(trn_type="TRN2")
    # x ships pre-padded (zero halo rows/cols) so no on-device memsets are
    # needed and every block load is one uniform strip DMA.
    xs = nc.dram_tensor("xs", [BL, CIN, H + 2, W + 2], BF16, kind="ExternalInput")
    # rw[r, i, p*O + o] = rotated filter bank, lhsT layout per tap
    rw = nc.dram_tensor("rw", [R, CIN, 9 * O], BF16, kind="ExternalInput")
    y = nc.dram_tensor("y", [BL, O, H, W], F32, kind="ExternalOutput")

    with TileContext(nc) as tc:
        with (
            tc.tile_pool(name="wrot", bufs=1) as rpool,
            tc.tile_pool(name="xio", bufs=1) as xpool,
            tc.tile_pool(name="accp", bufs=3) as apool,
            tc.tile_pool(name="psum", bufs=1, space="PSUM") as ppool,
        ):
            rotw = [
                rpool.tile([128, 9, O], BF16, name=f"rotw{r}", tag=f"rotw{r}")
                for r in range(R)
            ]

            # PE warm-up: matmuls on a scratch tile seeded by the very first
            # (tiny) DMA, so the dummies start as soon as the Tensor
            # sequencer boots (results land in the ps0 bank slot and are
            # overwritten by the first real start=True group).
            dum_lhs = rpool.tile([128, 128], BF16, name="dum_lhs", tag="dum")
            nc.sync.dma_start(out=dum_lhs[:, 0:64], in_=rw[0, :, 0:64])
            nc.sync.dma_start(out=dum_lhs[:, 64:128], in_=rw[0, :, 64:128])
            dum_ps = ppool.tile([128, 128], F32, name="dum_ps", tag="ps0")
            for _ in range(WARMUP):
                nc.tensor.matmul(
                    dum_ps[:, :], dum_lhs[:, :], dum_lhs[:, :],
                    start=True, stop=True,
                )

            # x staging: 3 persistent ping-pong buffers, fully overwritten by
            # each block's strip DMA (padding included), so no memsets ever.
            xst2 = [
                xpool.tile([128, RB + 2, W + 2], BF16, name=f"xst{i}", tag=f"xst{i}")
                for i in range(3)
            ]

            def load_x(g, b, blk, chunks=1, cuts=None):
                # DMA the block's padded input rows into the ping-pong
                # staging buffer.  `cuts`/`chunks` split the load so
                # downstream matmuls can start on the first rows before the
                # whole block has landed.
                xst = xst2[g % 3]
                r0 = blk * RB  # padded-row index of the block's top halo row
                nrows = RB + 2
                if cuts is None:
                    cuts = [nrows * k // chunks for k in range(chunks + 1)]
                for k in range(len(cuts) - 1):
                    a, c = cuts[k], cuts[k + 1]
                    nc.sync.dma_start(
                        out=xst[:, a:c, :],
                        in_=xs[b, :, r0 + a : r0 + c, :],
                    )
                return xst

            def conv_chunk(xmm, acc, r, store=None, s_groups=1, fine_tail=False):
                pst = [
                    ppool.tile([128, 4, W], F32, name=f"ps{s}", tag=f"ps{s}")
                    for s in range(NS)
                ]

                def emit_max_store(s, rows):
                    # rows: list of (row0, nrows) pieces within the subtile
                    for a, n in rows:
                        lo, hi = 4 * s + a, 4 * s + a + n
                        if r == 0:
                            nc.vector.tensor_copy(
                                acc[:, lo:hi, :], pst[s][:, a : a + n, :]
                            )
                        else:
                            nc.vector.tensor_tensor(
                                acc[:, lo:hi, :],
                                acc[:, lo:hi, :],
                                pst[s][:, a : a + n, :],
                                mybir.AluOpType.max,
                            )
                        if store is not None:
                            b, h0 = store
                            nc.sync.dma_start(
                                out=y[b, :, h0 + lo : h0 + hi, :],
                                in_=acc[:, lo:hi, :],
                            )

                def emit_group(ss):
                    for p in range(9):
                        ky, kx = divmod(p, 3)
                        lhsT = rotw[r][:, p, :]
                        for s in ss:
                            rhs = xmm[:, 4 * s + ky : 4 * s + ky + 4, kx : kx + W]
                            nc.tensor.matmul(
                                pst[s][:, :, :], lhsT, rhs,
                                start=(p == 0), stop=(p == 8),
                            )
                    for s in ss:
                        if fine_tail and s == NS - 1:
                            # drain the last subtile in 2-row pieces so the
                            # final store starts right behind the last matmul
                            emit_max_store(s, [(0, 2), (2, 2)])
                        else:
                            emit_max_store(s, [(0, 4)])

                per = NS // s_groups
                for k in range(s_groups):
                    emit_group(range(k * per, (k + 1) * per))

            # DMA issue order (the sync queue issues serially): rotation 0's
            # ky=0 taps (0-2) and the first 4 x rows go first so the very
            # first matmuls are unblocked right after the queues boot; the
            # remaining taps/rows/rotations follow interleaved by need-time.
            nc.sync.dma_start(
                out=rotw[0][:, 0:3, :].rearrange("i p o -> i (p o)"),
                in_=rw[0, :, 0 : 3 * O],
            )
            xst0 = xst2[0]
            def x0_chunk(a, c):
                nc.sync.dma_start(
                    out=xst0[:, a:c, :],
                    in_=xs[0, :, a:c, :],
                )
            x0_chunk(0, 6)
            nc.sync.dma_start(
                out=rotw[0][:, 3:9, :].rearrange("i p o -> i (p o)"),
                in_=rw[0, :, 3 * O : 9 * O],
            )
            x0_chunk(6, 10)
            x0_chunk(10, 14)
            nc.sync.dma_start(
                out=rotw[1][:, :, :].rearrange("i p o -> i (p o)"),
                in_=rw[1, :, :],
            )
            x0_chunk(14, 24)
            x0_chunk(24, 34)
            for r in range(2, R):
                nc.sync.dma_start(
                    out=rotw[r][:, :, :].rearrange("i p o -> i (p o)"),
                    in_=rw[r, :, :],
                )
            xmm_pre = [xst0]
            xmm_pre.append(load_x(1, 0, 1, chunks=2))
            xmm_pre.append(load_x(2, 0, 2, chunks=2))

            last_g = BL * NBLK - 1
            for g in range(BL * NBLK):
                b, blk = divmod(g, NBLK)
                xmm = xmm_pre[g] if g < 3 else load_x(g, b, blk)
                acc = apool.tile([128, RB, W], F32, name="acc", tag="acc")
                for r in range(R):
                    final = r == R - 1
                    # block 0 rotation 0 runs subtile-major so matmuls start
                    # as soon as the first x rows land; the very last chunk
                    # runs subtile-major so the final stores drain early.
                    sg = 8 if (g == 0 and r == 0) or (final and g == last_g) else 1
                    conv_chunk(
                        xmm, acc, r,
                        store=(b, blk * RB) if final else None,
                        s_groups=sg,
                        fine_tail=(final and g == last_g),
                    )
    nc.finalize()
    return nc


def _get_nc():
    if "v2" not in _NC_CACHE:
        _NC_CACHE["v2"] = _build()
    return _NC_CACHE["v2"]


def kernel(x, weight, rot_alpha):
    global LAST_RESULTS
    x = np.asarray(x, np.float32)
    weight = np.ascontiguousarray(np.asarray(weight, np.float32))
    rot_alpha = np.asarray(rot_alpha, np.float32)

    # Host-side filter rotation: rw[r, i, p, o] = sum_q M[r,p,q] * W[o*R+r, i, q]
    # in f32, then one cast to bf16 (same rounding boundary as the previous
    # on-device DVE mixing, so numerics are unchanged).
    M = _rot_mats(rot_alpha)
    wq = weight.reshape(O, R, CIN, 9).transpose(1, 2, 3, 0)  # (R, I, q, O)
    rot = np.einsum("rpq,riqo->ripo", M, wq)
    rw = np.ascontiguousarray(rot.reshape(R, CIN, 9 * O)).astype(
        ml_dtypes.bfloat16
    )
    xb = np.zeros((B, CIN, H + 2, W + 2), ml_dtypes.bfloat16)
    xb[:, :, 1 : H + 1, 1 : W + 1] = x.astype(ml_dtypes.bfloat16)

    nc = _get_nc()
    in_maps = [
        {"xs": np.ascontiguousarray(xb[c * BL : (c + 1) * BL]), "rw": rw}
        for c in range(NCORES)
    ]
    try:
        res = run_bass_kernel_spmd(nc, in_maps, list(range(NCORES)), trace=_TRACE)
    except Exception:
        # One retry (without tracing): a failed compile or an aborted run can
        # leave a NeuronCore transiently wedged; the next attempt recovers.
        res = run_bass_kernel_spmd(nc, in_maps, list(range(NCORES)), trace=False)
    LAST_RESULTS = res
    return np.concatenate([res.results[c]["y"] for c in range(NCORES)], axis=0)


# revision 18
# speedup vs baseline: 1.1996x; 1.0002x over previous
"""Equivariant rotation conv for Trainium2, 8-core batch-parallel.

Computes: rotate a (128*8, 128, 3, 3) filter bank by 8 data-dependent angles
(bilinear resampling), run a 3x3 same-padded conv of x (16,128,128,128) with
all 8*128 rotated filters, then max over the 8 rotations -> (16,128,128,128).

Sharding: data-parallel over batch, 2 images per core; the rotated filter
bank is replicated.  The rotation itself (a 9x9 tap-mixing matrix per
rotation, a pure function of the 8 rot_alpha scalars) is applied to the
filter bank on the HOST in f32 (10 MFLOP against the conv's 309 GFLOP) and
shipped pre-cast to bf16, so the device runs a pure conv+max pipeline:

  - per 32-row block: DMA the bf16 input rows (with zero halo kept
    persistent in SBUF) straight into ping-pong staging buffers,
  - the conv runs as 9 shifted PE matmuls in bf16 (K=Cin=128 partitions,
    N=512 spatial) accumulated in f32 PSUM, one PSUM bank per 4 output
    rows, 8 rotation chunks back to back,
  - a running elementwise max over the rotation chunks on DVE, with the
    final max fused with the per-slice output DMA.

The PE runs gap-free at ~218.3 ns per 512-column matmul (99.8% matrix
occupancy, measured: 512 cycles at 2.4 GHz + ~12 cycles fixed per-matmul
SBUF access latency); 4608 matmuls/core ≈ 1006 us is the silicon floor.
fp8 was evaluated and rejected: DoubleRow on real TRN2 runs at the same
ns-per-output-column as bf16 (2x MACs via K=256 packing), so the only
fp8 scheme that beats bf16 parity is uncorrected quantization, which
measures rel_l2 = 2.5e-2 against the 2e-2 gate.  The remaining time is
fixed overhead: ~7 us engine wake, ~3 us HAM clock ramp (absorbed by
warm-up dummies gated on the first tiny DMA), ~5.5 us drain + framework
epilogue; the head is minimized by DMA-ordering rotation-0/ky=0 taps and
the first 6 padded rows ahead of everything else.
"""

import numpy as np
import ml_dtypes


def _install_axon_hooks_shim():
    """Provide antenv.axon_hooks (NTFF profile hook) when the image's antenv
    lacks it, so run_bass_kernel_spmd(trace=True) works instead of crashing
    on import.  The hook drives NRT profiling via ctypes into the axon PJRT
    plugin, mirroring the boot-side installer."""
    import contextlib
    import ctypes
    import os
    import sys
    import types

    try:
        import antenv.axon_hooks  # noqa: F401

        return
    except ImportError:
        pass

    state = {"hook": None, "resolved": False}

    def _make_hook():
        so_path = os.environ.get("AXON_PJRT_SO", "/opt/axon/libaxon_pjrt.so")
        if not os.path.exists(so_path):
            return None
        lib = ctypes.CDLL(so_path)
        if not hasattr(lib, "axon_start_nrt_profile"):
            return None
        lib.axon_start_nrt_profile.argtypes = [
            ctypes.POINTER(ctypes.c_int64),
            ctypes.c_size_t,
        ]
        lib.axon_start_nrt_profile.restype = ctypes.c_int64
        lib.axon_stop_nrt_profile.argtypes = [ctypes.c_char_p]
        lib.axon_stop_nrt_profile.restype = ctypes.c_int64

        @contextlib.contextmanager
        def _hook(output_dir, device_ids):
            import jax

            jax.devices()
            if device_ids:
                ids = (ctypes.c_int64 * len(device_ids))(*device_ids)
                rc = lib.axon_start_nrt_profile(ids, len(device_ids))
            else:
                rc = lib.axon_start_nrt_profile(None, 0)
            if rc != 0:
                raise RuntimeError(f"axon_start_nrt_profile rc={rc}")
            try:
                yield
            finally:
                n = lib.axon_stop_nrt_profile(str(output_dir).encode())
                if n < 0:
                    raise RuntimeError(f"axon_stop_nrt_profile rc={n}")
                print(f"profile: {n} file(s) written to {output_dir}")

        return _hook

    mod = types.ModuleType("antenv.axon_hooks")

    def set_axon_ntff_profile_hook(h):
        state["hook"] = h
        state["resolved"] = True

    def get_axon_ntff_profile_hook():
        if not state["resolved"]:
            state["hook"] = _make_hook()
            state["resolved"] = True
        return state["hook"]

    mod.set_axon_ntff_profile_hook = set_axon_ntff_profile_hook
    mod.get_axon_ntff_profile_hook = get_axon_ntff_profile_hook
    sys.modules["antenv.axon_hooks"] = mod


_install_axon_hooks_shim()

import concourse.bass as bass  # noqa: E402,F401
import concourse.mybir as mybir  # noqa: E402
from concourse import bacc  # noqa: E402
from concourse.bass_utils import run_bass_kernel_spmd  # noqa: E402
from concourse.tile import TileContext  # noqa: E402

F32 = mybir.dt.float32
BF16 = mybir.dt.bfloat16

B, CIN, H, W = 16, 128, 128, 128
R, O, K = 8, 128, 3
NCORES = 8
BL = B // NCORES  # images per core
RB = 32           # output rows per block
NS = RB // 4      # psum subtiles (4 rows = 512 cols) per block
NBLK = H // RB

# PE warm-up matmuls before the first real work (HAM clock ramp + keeps the
# PE busy while the first weight/x DMAs land).  28 x ~107ns low-clock dummies
# from ~7.6us give ~3us of continuous PE execution, so the clock is fully
# ramped right when the first x/weight DMA semaphores become visible (~10.7us).
WARMUP = 28

_TRACE = False
LAST_RESULTS = None
_NC_CACHE = {}


def _rot_mats(rot_alpha):
    """Per-rotation 9x9 bilinear resampling matrices, matching the reference
    F.grid_sample(align_corners=True, zeros) tap logic exactly.

    M[r, p, q]: coefficient of original tap q = (qy*3+qx) in rotated tap
    p = (py*3+px)."""
    M = np.zeros((R, 9, 9), np.float64)
    lin = np.linspace(-1.0, 1.0, K)
    for r in range(R):
        ang = float(rot_alpha[r]) * (np.pi / 4.0) * r
        c, s = np.cos(ang), np.sin(ang)
        for a in range(K):          # output row (gy = lin[a])
            for b in range(K):      # output col (gx = lin[b])
                gx, gy = lin[b], lin[a]
                xs = c * gx - s * gy
                ys = s * gx + c * gy
                ix = (xs + 1.0) * 0.5 * (K - 1)
                iy = (ys + 1.0) * 0.5 * (K - 1)
                x0 = int(np.floor(ix))
                y0 = int(np.floor(iy))
                wx, wy = ix - x0, iy - y0
                p = a * K + b
                for yi, xi, wt in (
                    (y0, x0, (1 - wy) * (1 - wx)),
                    (y0, x0 + 1, (1 - wy) * wx),
                    (y0 + 1, x0, wy * (1 - wx)),
                    (y0 + 1, x0 + 1, wy * wx),
                ):
                    if 0 <= yi < K and 0 <= xi < K:
                        M[r, p, yi * K + xi] += wt
    return M.astype(np.float32)


def _build():
    nc = bacc.Bacc(trn_type="TRN2")
    # x ships pre-padded (zero halo rows/cols) so no on-device memsets are
    # needed and every block load is one uniform strip DMA.
    xs = nc.dram_tensor("xs", [BL, CIN, H + 2, W + 2], BF16, kind="ExternalInput")
    # rw[r, i, p*O + o] = rotated filter bank, lhsT layout per tap
    rw = nc.dram_tensor("rw", [R, CIN, 9 * O], BF16, kind="ExternalInput")
    y = nc.dram_tensor("y", [BL, O, H, W], F32, kind="ExternalOutput")

    with TileContext(nc) as tc:
        with (
            tc.tile_pool(name="wrot", bufs=1) as rpool,
            tc.tile_pool(name="xio", bufs=1) as xpool,
            tc.tile_pool(name="accp", bufs=3) as apool,
            tc.tile_pool(name="psum", bufs=1, space="PSUM") as ppool,
        ):
            rotw = [
                rpool.tile([128, 9, O], BF16, name=f"rotw{r}", tag=f"rotw{r}")
                for r in range(R)
            ]

            # PE warm-up: matmuls on a scratch tile seeded by a gpsimd memset
            # (engine-to-engine semaphores are visible in ~1.5us, vs ~4us for
            # DMA-completion semaphores, and gpsimd gates no DMA queue here).
            # Results land in the ps0 bank slot and are overwritten by the
            # first real start=True group.
            dum_lhs = rpool.tile([128, 128], BF16, name="dum_lhs", tag="dum")
            nc.gpsimd.memset(dum_lhs[:, :], 0.0)
            dum_ps = ppool.tile([128, 128], F32, name="dum_ps", tag="ps0")
            for _ in range(WARMUP):
                nc.tensor.matmul(
                    dum_ps[:, :], dum_lhs[:, :], dum_lhs[:, :],
                    start=True, stop=True,
                )

            # x staging: 3 persistent ping-pong buffers, fully overwritten by
            # each block's strip DMA (padding included), so no memsets ever.
            xst2 = [
                xpool.tile([128, RB + 2, W + 2], BF16, name=f"xst{i}", tag=f"xst{i}")
                for i in range(3)
            ]

            def load_x(g, b, blk, chunks=1, cuts=None):
                # DMA the block's padded input rows into the ping-pong
                # staging buffer.  `cuts`/`chunks` split the load so
                # downstream matmuls can start on the first rows before the
                # whole block has landed.
                xst = xst2[g % 3]
                r0 = blk * RB  # padded-row index of the block's top halo row
                nrows = RB + 2
                if cuts is None:
                    cuts = [nrows * k // chunks for k in range(chunks + 1)]
                for k in range(len(cuts) - 1):
                    a, c = cuts[k], cuts[k + 1]
                    nc.sync.dma_start(
                        out=xst[:, a:c, :],
                        in_=xs[b, :, r0 + a : r0 + c, :],
                    )
                return xst

            def conv_chunk(xmm, acc, r, store=None, s_groups=1, fine_tail=False):
                pst = [
                    ppool.tile([128, 4, W], F32, name=f"ps{s}", tag=f"ps{s}")
                    for s in range(NS)
                ]

                def emit_max_store(s, rows):
                    # rows: list of (row0, nrows) pieces within the subtile
                    for a, n in rows:
                        lo, hi = 4 * s + a, 4 * s + a + n
                        if r == 0:
                            nc.vector.tensor_copy(
                                acc[:, lo:hi, :], pst[s][:, a : a + n, :]
                            )
                        else:
                            nc.vector.tensor_tensor(
                                acc[:, lo:hi, :],
                                acc[:, lo:hi, :],
                                pst[s][:, a : a + n, :],
                                mybir.AluOpType.max,
                            )
                        if store is not None:
                            b, h0 = store
                            nc.sync.dma_start(
                                out=y[b, :, h0 + lo : h0 + hi, :],
                                in_=acc[:, lo:hi, :],
                            )

                def emit_group(ss):
                    for p in range(9):
                        ky, kx = divmod(p, 3)
                        lhsT = rotw[r][:, p, :]
                        for s in ss:
                            rhs = xmm[:, 4 * s + ky : 4 * s + ky + 4, kx : kx + W]
                            nc.tensor.matmul(
                                pst[s][:, :, :], lhsT, rhs,
                                start=(p == 0), stop=(p == 8),
                            )
                    for s in ss:
                        if fine_tail and s == NS - 1:
                            # drain the last subtile in 2-row pieces so the
                            # final store starts right behind the last matmul
                            emit_max_store(s, [(0, 2), (2, 2)])
                        else:
                            emit_max_store(s, [(0, 4)])

                per = NS // s_groups
                for k in range(s_groups):
                    emit_group(range(k * per, (k + 1) * per))

            # DMA issue order (the sync queue issues serially): rotation 0's
            # ky=0 taps (0-2) and the first 4 x rows go first so the very
            # first matmuls are unblocked right after the queues boot; the
            # remaining taps/rows/rotations follow interleaved by need-time.
            nc.sync.dma_start(
                out=rotw[0][:, 0:3, :].rearrange("i p o -> i (p o)"),
                in_=rw[0, :, 0 : 3 * O],
            )
            xst0 = xst2[0]
            def x0_chunk(a, c):
                nc.sync.dma_start(
                    out=xst0[:, a:c, :],
                    in_=xs[0, :, a:c, :],
                )
            x0_chunk(0, 6)
            nc.sync.dma_start(
                out=rotw[0][:, 3:9, :].rearrange("i p o -> i (p o)"),
                in_=rw[0, :, 3 * O : 9 * O],
            )
            x0_chunk(6, 10)
            x0_chunk(10, 14)
            nc.sync.dma_start(
                out=rotw[1][:, :, :].rearrange("i p o -> i (p o)"),
                in_=rw[1, :, :],
            )
            x0_chunk(14, 24)
            x0_chunk(24, 34)
            for r in range(2, R):
                nc.sync.dma_start(
                    out=rotw[r][:, :, :].rearrange("i p o -> i (p o)"),
                    in_=rw[r, :, :],
                )
            xmm_pre = [xst0]
            xmm_pre.append(load_x(1, 0, 1, chunks=2))
            xmm_pre.append(load_x(2, 0, 2, chunks=2))

            last_g = BL * NBLK - 1
            for g in range(BL * NBLK):
                b, blk = divmod(g, NBLK)
                xmm = xmm_pre[g] if g < 3 else load_x(g, b, blk)
                acc = apool.tile([128, RB, W], F32, name="acc", tag="acc")
                for r in range(R):
                    final = r == R - 1
                    # block 0 rotation 0 runs subtile-major so matmuls start
                    # as soon as the first x rows land; the very last chunk
                    # runs subtile-major so the final stores drain early.
                    sg = 8 if (g == 0 and r == 0) or (final and g == last_g) else 1
                    conv_chunk(
                        xmm, acc, r,
                        store=(b, blk * RB) if final else None,
                        s_groups=sg,
                        fine_tail=(final and g == last_g),
                    )
    nc.finalize()
    return nc


def _get_nc():
    if "v2" not in _NC_CACHE:
        _NC_CACHE["v2"] = _build()
    return _NC_CACHE["v2"]


def kernel(x, weight, rot_alpha):
    global LAST_RESULTS
    x = np.asarray(x, np.float32)
    weight = np.ascontiguousarray(np.asarray(weight, np.float32))
    rot_alpha = np.asarray(rot_alpha, np.float32)

    # Host-side filter rotation: rw[r, i, p, o] = sum_q M[r,p,q] * W[o*R+r, i, q]
    # in f32, then one cast to bf16 (same rounding boundary as the previous
    # on-device DVE mixing, so numerics are unchanged).
    M = _rot_mats(rot_alpha)
    wq = weight.reshape(O, R, CIN, 9).transpose(1, 2, 3, 0)  # (R, I, q, O)
    rot = np.einsum("rpq,riqo->ripo", M, wq)
    rw = np.ascontiguousarray(rot.reshape(R, CIN, 9 * O)).astype(
        ml_dtypes.bfloat16
    )
    xb = np.zeros((B, CIN, H + 2, W + 2), ml_dtypes.bfloat16)
    xb[:, :, 1 : H + 1, 1 : W + 1] = x.astype(ml_dtypes.bfloat16)

    nc = _get_nc()
    in_maps = [
        {"xs": np.ascontiguousarray(xb[c * BL : (c + 1) * BL]), "rw": rw}
        for c in range(NCORES)
    ]
    try:
        res = run_bass_kernel_spmd(nc, in_maps, list(range(NCORES)), trace=_TRACE)
    except Exception:
        # One retry (without tracing): a failed compile or an aborted run can
        # leave a NeuronCore transiently wedged; the next attempt recovers.
        res = run_bass_kernel_spmd(nc, in_maps, list(range(NCORES)), trace=False)
    LAST_RESULTS = res
    return np.concatenate([res.results[c]["y"] for c in range(NCORES)], axis=0)


# revision 21
# speedup vs baseline: 1.2051x; 1.0046x over previous
"""Equivariant rotation conv for Trainium2, 8-core batch-parallel.

Computes: rotate a (128*8, 128, 3, 3) filter bank by 8 data-dependent angles
(bilinear resampling), run a 3x3 same-padded conv of x (16,128,128,128) with
all 8*128 rotated filters, then max over the 8 rotations -> (16,128,128,128).

Sharding: data-parallel over batch, 2 images per core; the rotated filter
bank is replicated.  The rotation itself (a 9x9 tap-mixing matrix per
rotation, a pure function of the 8 rot_alpha scalars) is applied to the
filter bank on the HOST in f32 (10 MFLOP against the conv's 309 GFLOP) and
shipped pre-cast to bf16, so the device runs a pure conv+max pipeline:

  - per 32-row block: DMA the bf16 input rows (with zero halo kept
    persistent in SBUF) straight into ping-pong staging buffers,
  - the conv runs as 9 shifted PE matmuls in bf16 (K=Cin=128 partitions,
    N=512 spatial) accumulated in f32 PSUM, one PSUM bank per 4 output
    rows, 8 rotation chunks back to back,
  - a running elementwise max over the rotation chunks on DVE, with the
    final max fused with the per-slice output DMA.

The PE runs gap-free at ~218.3 ns per 512-column matmul (99.8% matrix
occupancy, measured: 512 cycles at 2.4 GHz + ~12 cycles fixed per-matmul
SBUF access latency); 4608 matmuls/core ≈ 1006 us is the silicon floor.
fp8 was evaluated and rejected: DoubleRow on real TRN2 runs at the same
ns-per-output-column as bf16 (2x MACs via K=256 packing), so the only
fp8 scheme that beats bf16 parity is uncorrected quantization, which
measures rel_l2 = 2.5e-2 against the 2e-2 gate.  The remaining time is
fixed overhead: ~7 us engine wake, ~3 us HAM clock ramp (absorbed by
warm-up dummies gated on the first tiny DMA), ~5.5 us drain + framework
epilogue; the head is minimized by DMA-ordering rotation-0/ky=0 taps and
the first 6 padded rows ahead of everything else.
"""

import numpy as np
import ml_dtypes


def _install_axon_hooks_shim():
    """Provide antenv.axon_hooks (NTFF profile hook) when the image's antenv
    lacks it, so run_bass_kernel_spmd(trace=True) works instead of crashing
    on import.  The hook drives NRT profiling via ctypes into the axon PJRT
    plugin, mirroring the boot-side installer."""
    import contextlib
    import ctypes
    import os
    import sys
    import types

    try:
        import antenv.axon_hooks  # noqa: F401

        return
    except ImportError:
        pass

    state = {"hook": None, "resolved": False}

    def _make_hook():
        so_path = os.environ.get("AXON_PJRT_SO", "/opt/axon/libaxon_pjrt.so")
        if not os.path.exists(so_path):
            return None
        lib = ctypes.CDLL(so_path)
        if not hasattr(lib, "axon_start_nrt_profile"):
            return None
        lib.axon_start_nrt_profile.argtypes = [
            ctypes.POINTER(ctypes.c_int64),
            ctypes.c_size_t,
        ]
        lib.axon_start_nrt_profile.restype = ctypes.c_int64
        lib.axon_stop_nrt_profile.argtypes = [ctypes.c_char_p]
        lib.axon_stop_nrt_profile.restype = ctypes.c_int64

        @contextlib.contextmanager
        def _hook(output_dir, device_ids):
            import jax

            jax.devices()
            if device_ids:
                ids = (ctypes.c_int64 * len(device_ids))(*device_ids)
                rc = lib.axon_start_nrt_profile(ids, len(device_ids))
            else:
                rc = lib.axon_start_nrt_profile(None, 0)
            if rc != 0:
                raise RuntimeError(f"axon_start_nrt_profile rc={rc}")
            try:
                yield
            finally:
                n = lib.axon_stop_nrt_profile(str(output_dir).encode())
                if n < 0:
                    raise RuntimeError(f"axon_stop_nrt_profile rc={n}")
                print(f"profile: {n} file(s) written to {output_dir}")

        return _hook

    mod = types.ModuleType("antenv.axon_hooks")

    def set_axon_ntff_profile_hook(h):
        state["hook"] = h
        state["resolved"] = True

    def get_axon_ntff_profile_hook():
        if not state["resolved"]:
            state["hook"] = _make_hook()
            state["resolved"] = True
        return state["hook"]

    mod.set_axon_ntff_profile_hook = set_axon_ntff_profile_hook
    mod.get_axon_ntff_profile_hook = get_axon_ntff_profile_hook
    sys.modules["antenv.axon_hooks"] = mod


_install_axon_hooks_shim()

import concourse.bass as bass  # noqa: E402,F401
import concourse.mybir as mybir  # noqa: E402
from concourse import bacc  # noqa: E402
from concourse.bass_utils import run_bass_kernel_spmd  # noqa: E402
from concourse.tile import TileContext  # noqa: E402

F32 = mybir.dt.float32
BF16 = mybir.dt.bfloat16

B, CIN, H, W = 16, 128, 128, 128
R, O, K = 8, 128, 3
NCORES = 8
BL = B // NCORES  # images per core
RB = 32           # output rows per block
NS = RB // 4      # psum subtiles (4 rows = 512 cols) per block
NBLK = H // RB

# PE warm-up matmuls before the first real work (HAM clock ramp + keeps the
# PE busy while the first weight/x DMAs land).  28 x ~107ns low-clock dummies
# from ~7.6us give ~3us of continuous PE execution, so the clock is fully
# ramped right when the first x/weight DMA semaphores become visible (~10.7us).
WARMUP = 28

_TRACE = False
LAST_RESULTS = None
_NC_CACHE = {}


def _rot_mats(rot_alpha):
    """Per-rotation 9x9 bilinear resampling matrices, matching the reference
    F.grid_sample(align_corners=True, zeros) tap logic exactly.

    M[r, p, q]: coefficient of original tap q = (qy*3+qx) in rotated tap
    p = (py*3+px)."""
    M = np.zeros((R, 9, 9), np.float64)
    lin = np.linspace(-1.0, 1.0, K)
    for r in range(R):
        ang = float(rot_alpha[r]) * (np.pi / 4.0) * r
        c, s = np.cos(ang), np.sin(ang)
        for a in range(K):          # output row (gy = lin[a])
            for b in range(K):      # output col (gx = lin[b])
                gx, gy = lin[b], lin[a]
                xs = c * gx - s * gy
                ys = s * gx + c * gy
                ix = (xs + 1.0) * 0.5 * (K - 1)
                iy = (ys + 1.0) * 0.5 * (K - 1)
                x0 = int(np.floor(ix))
                y0 = int(np.floor(iy))
                wx, wy = ix - x0, iy - y0
                p = a * K + b
                for yi, xi, wt in (
                    (y0, x0, (1 - wy) * (1 - wx)),
                    (y0, x0 + 1, (1 - wy) * wx),
                    (y0 + 1, x0, wy * (1 - wx)),
                    (y0 + 1, x0 + 1, wy * wx),
                ):
                    if 0 <= yi < K and 0 <= xi < K:
                        M[r, p, yi * K + xi] += wt
    return M.astype(np.float32)


def _build():
    nc = bacc.Bacc(trn_type="TRN2")
    # x ships pre-padded (zero halo rows/cols) so no on-device memsets are
    # needed and every block load is one uniform strip DMA.
    xs = nc.dram_tensor("xs", [BL, CIN, H + 2, W + 2], BF16, kind="ExternalInput")
    # rw[r, i, p*O + o] = rotated filter bank, lhsT layout per tap
    rw = nc.dram_tensor("rw", [R, CIN, 9 * O], BF16, kind="ExternalInput")
    y = nc.dram_tensor("y", [BL, O, H, W], F32, kind="ExternalOutput")

    with TileContext(nc) as tc:
        with (
            tc.tile_pool(name="wrot", bufs=1) as rpool,
            tc.tile_pool(name="xio", bufs=1) as xpool,
            tc.tile_pool(name="accp", bufs=3) as apool,
            tc.tile_pool(name="psum", bufs=1, space="PSUM") as ppool,
        ):
            rotw = [
                rpool.tile([128, 9, O], BF16, name=f"rotw{r}", tag=f"rotw{r}")
                for r in range(R)
            ]

            # PE warm-up: matmuls on a scratch tile seeded by a gpsimd memset
            # (engine-to-engine semaphores are visible in ~1.5us, vs ~4us for
            # DMA-completion semaphores, and gpsimd gates no DMA queue here).
            # Results land in the ps0 bank slot and are overwritten by the
            # first real start=True group.
            dum_lhs = rpool.tile([128, 128], BF16, name="dum_lhs", tag="dum")
            nc.gpsimd.memset(dum_lhs[:, :], 0.0)
            dum_ps = ppool.tile([128, 128], F32, name="dum_ps", tag="ps0")
            for _ in range(WARMUP):
                nc.tensor.matmul(
                    dum_ps[:, :], dum_lhs[:, :], dum_lhs[:, :],
                    start=True, stop=True,
                )

            # x staging: 3 persistent ping-pong buffers, fully overwritten by
            # each block's strip DMA (padding included), so no memsets ever.
            xst2 = [
                xpool.tile([128, RB + 2, W + 2], BF16, name=f"xst{i}", tag=f"xst{i}")
                for i in range(3)
            ]

            def load_x(g, b, blk, chunks=1, cuts=None):
                # DMA the block's padded input rows into the ping-pong
                # staging buffer.  `cuts`/`chunks` split the load so
                # downstream matmuls can start on the first rows before the
                # whole block has landed.
                xst = xst2[g % 3]
                r0 = blk * RB  # padded-row index of the block's top halo row
                nrows = RB + 2
                if cuts is None:
                    cuts = [nrows * k // chunks for k in range(chunks + 1)]
                for k in range(len(cuts) - 1):
                    a, c = cuts[k], cuts[k + 1]
                    nc.sync.dma_start(
                        out=xst[:, a:c, :],
                        in_=xs[b, :, r0 + a : r0 + c, :],
                    )
                return xst

            # Tap emission order: the center tap (ky=1, kx=1) goes first so
            # the start=True matmul covers the full PSUM region; edge taps
            # can then shrink their APs to skip columns/rows that only
            # multiply the zero halo (their contribution is exactly zero).
            TAPS = [4, 0, 1, 2, 3, 5, 6, 7, 8]

            def conv_chunk(xmm, acc, r, blk, store=None, s_groups=1,
                           fine_tail=False):
                top = blk == 0
                bot = blk == NBLK - 1
                pst = [
                    ppool.tile([128, 4, W], F32, name=f"ps{s}", tag=f"ps{s}")
                    for s in range(NS)
                ]

                def emit_max_store(s, rows):
                    # rows: list of (row0, nrows) pieces within the subtile
                    for a, n in rows:
                        lo, hi = 4 * s + a, 4 * s + a + n
                        if r == 0:
                            nc.vector.tensor_copy(
                                acc[:, lo:hi, :], pst[s][:, a : a + n, :]
                            )
                        else:
                            nc.vector.tensor_tensor(
                                acc[:, lo:hi, :],
                                acc[:, lo:hi, :],
                                pst[s][:, a : a + n, :],
                                mybir.AluOpType.max,
                            )
                        if store is not None:
                            b, h0 = store
                            nc.sync.dma_start(
                                out=y[b, :, h0 + lo : h0 + hi, :],
                                in_=acc[:, lo:hi, :],
                            )

                def emit_group(ss):
                    for p in TAPS:
                        ky, kx = divmod(p, 3)
                        lhsT = rotw[r][:, p, :]
                        # out col c <- xmm col c+kx; skip the column that
                        # only reads the zero halo (kx=0: out col 0, kx=2:
                        # out col W-1)
                        if kx == 0:
                            c0, ncol, x0 = 1, W - 1, 1
                        elif kx == 2:
                            c0, ncol, x0 = 0, W - 1, 2
                        else:
                            c0, ncol, x0 = 0, W, 1
                        for s in ss:
                            # skip the output row that only reads the halo
                            # row at the image top/bottom
                            r0, nr = 0, 4
                            if top and s == 0 and ky == 0:
                                r0, nr = 1, 3
                            if bot and s == NS - 1 and ky == 2:
                                nr = 3
                            rhs = xmm[
                                :,
                                4 * s + ky + r0 : 4 * s + ky + r0 + nr,
                                x0 : x0 + ncol,
                            ]
                            nc.tensor.matmul(
                                pst[s][:, r0 : r0 + nr, c0 : c0 + ncol],
                                lhsT, rhs,
                                start=(p == 4), stop=(p == 8),
                            )
                    for s in ss:
                        if fine_tail and s == NS - 1:
                            # drain the last subtile in 2-row pieces so the
                            # final store starts right behind the last matmul
                            emit_max_store(s, [(0, 2), (2, 2)])
                        else:
                            emit_max_store(s, [(0, 4)])

                per = NS // s_groups
                for k in range(s_groups):
                    emit_group(range(k * per, (k + 1) * per))

            # DMA issue order (the sync queue issues serially): rotation 0's
            # ky=0 taps (0-2) and the first 4 x rows go first so the very
            # first matmuls are unblocked right after the queues boot; the
            # remaining taps/rows/rotations follow interleaved by need-time.
            nc.sync.dma_start(
                out=rotw[0][:, 0:3, :].rearrange("i p o -> i (p o)"),
                in_=rw[0, :, 0 : 3 * O],
            )
            xst0 = xst2[0]
            def x0_chunk(a, c):
                nc.sync.dma_start(
                    out=xst0[:, a:c, :],
                    in_=xs[0, :, a:c, :],
                )
            x0_chunk(0, 6)
            nc.sync.dma_start(
                out=rotw[0][:, 3:9, :].rearrange("i p o -> i (p o)"),
                in_=rw[0, :, 3 * O : 9 * O],
            )
            x0_chunk(6, 10)
            x0_chunk(10, 14)
            nc.sync.dma_start(
                out=rotw[1][:, :, :].rearrange("i p o -> i (p o)"),
                in_=rw[1, :, :],
            )
            x0_chunk(14, 24)
            x0_chunk(24, 34)
            for r in range(2, R):
                nc.sync.dma_start(
                    out=rotw[r][:, :, :].rearrange("i p o -> i (p o)"),
                    in_=rw[r, :, :],
                )
            xmm_pre = [xst0]
            xmm_pre.append(load_x(1, 0, 1, chunks=2))
            xmm_pre.append(load_x(2, 0, 2, chunks=2))

            last_g = BL * NBLK - 1
            for g in range(BL * NBLK):
                b, blk = divmod(g, NBLK)
                xmm = xmm_pre[g] if g < 3 else load_x(g, b, blk)
                acc = apool.tile([128, RB, W], F32, name="acc", tag="acc")
                for r in range(R):
                    final = r == R - 1
                    # block 0 rotation 0 runs subtile-major so matmuls start
                    # as soon as the first x rows land; the very last chunk
                    # runs subtile-major so the final stores drain early.
                    sg = 8 if (g == 0 and r == 0) or (final and g == last_g) else 1
                    conv_chunk(
                        xmm, acc, r, blk,
                        store=(b, blk * RB) if final else None,
                        s_groups=sg,
                        fine_tail=(final and g == last_g),
                    )
    nc.finalize()
    return nc


def _get_nc():
    if "v2" not in _NC_CACHE:
        _NC_CACHE["v2"] = _build()
    return _NC_CACHE["v2"]


def kernel(x, weight, rot_alpha):
    global LAST_RESULTS
    x = np.asarray(x, np.float32)
    weight = np.ascontiguousarray(np.asarray(weight, np.float32))
    rot_alpha = np.asarray(rot_alpha, np.float32)

    # Host-side filter rotation: rw[r, i, p, o] = sum_q M[r,p,q] * W[o*R+r, i, q]
    # in f32, then one cast to bf16 (same rounding boundary as the previous
    # on-device DVE mixing, so numerics are unchanged).
    M = _rot_mats(rot_alpha)
    wq = weight.reshape(O, R, CIN, 9).transpose(1, 2, 3, 0)  # (R, I, q, O)
    rot = np.einsum("rpq,riqo->ripo", M, wq)
    rw = np.ascontiguousarray(rot.reshape(R, CIN, 9 * O)).astype(
        ml_dtypes.bfloat16
    )
    xb = np.zeros((B, CIN, H + 2, W + 2), ml_dtypes.bfloat16)
    xb[:, :, 1 : H + 1, 1 : W + 1] = x.astype(ml_dtypes.bfloat16)

    nc = _get_nc()
    in_maps = [
        {"xs": np.ascontiguousarray(xb[c * BL : (c + 1) * BL]), "rw": rw}
        for c in range(NCORES)
    ]
    try:
        res = run_bass_kernel_spmd(nc, in_maps, list(range(NCORES)), trace=_TRACE)
    except Exception:
        # One retry (without tracing): a failed compile or an aborted run can
        # leave a NeuronCore transiently wedged; the next attempt recovers.
        res = run_bass_kernel_spmd(nc, in_maps, list(range(NCORES)), trace=False)
    LAST_RESULTS = res
    return np.concatenate([res.results[c]["y"] for c in range(NCORES)], axis=0)


# revision 23
# speedup vs baseline: 1.2103x; 1.0043x over previous
"""Equivariant rotation conv for Trainium2, 8-core batch-parallel.

Computes: rotate a (128*8, 128, 3, 3) filter bank by 8 data-dependent angles
(bilinear resampling), run a 3x3 same-padded conv of x (16,128,128,128) with
all 8*128 rotated filters, then max over the 8 rotations -> (16,128,128,128).

Sharding: data-parallel over batch, 2 images per core; the rotated filter
bank is replicated.  The rotation itself (a 9x9 tap-mixing matrix per
rotation, a pure function of the 8 rot_alpha scalars) is applied to the
filter bank on the HOST in f32 (10 MFLOP against the conv's 309 GFLOP) and
shipped pre-cast to bf16, so the device runs a pure conv+max pipeline:

  - per 32-row block: DMA the bf16 input rows (with zero halo kept
    persistent in SBUF) straight into ping-pong staging buffers,
  - the conv runs as 9 shifted PE matmuls in bf16 (K=Cin=128 partitions,
    N=512 spatial) accumulated in f32 PSUM, one PSUM bank per 4 output
    rows, 8 rotation chunks back to back,
  - a running elementwise max over the rotation chunks on DVE, with the
    final max fused with the per-slice output DMA.

The PE runs gap-free at ~218.3 ns per 512-column matmul (99.8% matrix
occupancy, measured: 512 cycles at 2.4 GHz + ~12 cycles fixed per-matmul
SBUF access latency).  Edge taps shrink their matmul APs to skip the
~24.5k columns/core that only multiply the zero halo (kx=0 skips output
col 0, kx=2 col W-1, boundary-block ky taps skip a whole row); the center
tap is emitted first so the start=True matmul still covers the full PSUM
region.  ~4600 matmuls/core ≈ 1007 us is the silicon floor.
fp8 was evaluated and rejected: DoubleRow on real TRN2 runs at the same
ns-per-output-column as bf16 (2x MACs via K=256 packing), so the only
fp8 scheme that beats bf16 parity is uncorrected quantization, which
measures rel_l2 = 2.5e-2 against the 2e-2 gate.  The remaining time is
fixed overhead: ~7 us engine wake, ~3 us HAM clock ramp (absorbed by
warm-up dummies gated on the first tiny DMA), ~5.5 us drain + framework
epilogue; the head is minimized by DMA-ordering rotation-0/ky=0 taps and
the first 6 padded rows ahead of everything else.
"""

import numpy as np
import ml_dtypes


def _install_axon_hooks_shim():
    """Provide antenv.axon_hooks (NTFF profile hook) when the image's antenv
    lacks it, so run_bass_kernel_spmd(trace=True) works instead of crashing
    on import.  The hook drives NRT profiling via ctypes into the axon PJRT
    plugin, mirroring the boot-side installer."""
    import contextlib
    import ctypes
    import os
    import sys
    import types

    try:
        import antenv.axon_hooks  # noqa: F401

        return
    except ImportError:
        pass

    state = {"hook": None, "resolved": False}

    def _make_hook():
        so_path = os.environ.get("AXON_PJRT_SO", "/opt/axon/libaxon_pjrt.so")
        if not os.path.exists(so_path):
            return None
        lib = ctypes.CDLL(so_path)
        if not hasattr(lib, "axon_start_nrt_profile"):
            return None
        lib.axon_start_nrt_profile.argtypes = [
            ctypes.POINTER(ctypes.c_int64),
            ctypes.c_size_t,
        ]
        lib.axon_start_nrt_profile.restype = ctypes.c_int64
        lib.axon_stop_nrt_profile.argtypes = [ctypes.c_char_p]
        lib.axon_stop_nrt_profile.restype = ctypes.c_int64

        @contextlib.contextmanager
        def _hook(output_dir, device_ids):
            import jax

            jax.devices()
            if device_ids:
                ids = (ctypes.c_int64 * len(device_ids))(*device_ids)
                rc = lib.axon_start_nrt_profile(ids, len(device_ids))
            else:
                rc = lib.axon_start_nrt_profile(None, 0)
            if rc != 0:
                raise RuntimeError(f"axon_start_nrt_profile rc={rc}")
            try:
                yield
            finally:
                n = lib.axon_stop_nrt_profile(str(output_dir).encode())
                if n < 0:
                    raise RuntimeError(f"axon_stop_nrt_profile rc={n}")
                print(f"profile: {n} file(s) written to {output_dir}")

        return _hook

    mod = types.ModuleType("antenv.axon_hooks")

    def set_axon_ntff_profile_hook(h):
        state["hook"] = h
        state["resolved"] = True

    def get_axon_ntff_profile_hook():
        if not state["resolved"]:
            state["hook"] = _make_hook()
            state["resolved"] = True
        return state["hook"]

    mod.set_axon_ntff_profile_hook = set_axon_ntff_profile_hook
    mod.get_axon_ntff_profile_hook = get_axon_ntff_profile_hook
    sys.modules["antenv.axon_hooks"] = mod


_install_axon_hooks_shim()

import concourse.bass as bass  # noqa: E402,F401
import concourse.mybir as mybir  # noqa: E402
from concourse import bacc  # noqa: E402
from concourse.bass_utils import run_bass_kernel_spmd  # noqa: E402
from concourse.tile import TileContext  # noqa: E402

F32 = mybir.dt.float32
BF16 = mybir.dt.bfloat16

B, CIN, H, W = 16, 128, 128, 128
R, O, K = 8, 128, 3
NCORES = 8
BL = B // NCORES  # images per core
RB = 32           # output rows per block
NS = RB // 4      # psum subtiles (4 rows = 512 cols) per block
NBLK = H // RB

# PE warm-up matmuls before the first real work (HAM clock ramp + keeps the
# PE busy while the first weight/x DMAs land).  28 x ~107ns low-clock dummies
# from ~7.6us give ~3us of continuous PE execution, so the clock is fully
# ramped right when the first x/weight DMA semaphores become visible (~10.7us).
WARMUP = 28

_TRACE = False
LAST_RESULTS = None
_NC_CACHE = {}


def _rot_mats(rot_alpha):
    """Per-rotation 9x9 bilinear resampling matrices, matching the reference
    F.grid_sample(align_corners=True, zeros) tap logic exactly.

    M[r, p, q]: coefficient of original tap q = (qy*3+qx) in rotated tap
    p = (py*3+px)."""
    M = np.zeros((R, 9, 9), np.float64)
    lin = np.linspace(-1.0, 1.0, K)
    for r in range(R):
        ang = float(rot_alpha[r]) * (np.pi / 4.0) * r
        c, s = np.cos(ang), np.sin(ang)
        for a in range(K):          # output row (gy = lin[a])
            for b in range(K):      # output col (gx = lin[b])
                gx, gy = lin[b], lin[a]
                xs = c * gx - s * gy
                ys = s * gx + c * gy
                ix = (xs + 1.0) * 0.5 * (K - 1)
                iy = (ys + 1.0) * 0.5 * (K - 1)
                x0 = int(np.floor(ix))
                y0 = int(np.floor(iy))
                wx, wy = ix - x0, iy - y0
                p = a * K + b
                for yi, xi, wt in (
                    (y0, x0, (1 - wy) * (1 - wx)),
                    (y0, x0 + 1, (1 - wy) * wx),
                    (y0 + 1, x0, wy * (1 - wx)),
                    (y0 + 1, x0 + 1, wy * wx),
                ):
                    if 0 <= yi < K and 0 <= xi < K:
                        M[r, p, yi * K + xi] += wt
    return M.astype(np.float32)


def _build():
    nc = bacc.Bacc(trn_type="TRN2")
    # x ships pre-padded (zero halo rows/cols) so no on-device memsets are
    # needed and every block load is one uniform strip DMA.
    xs = nc.dram_tensor("xs", [BL, CIN, H + 2, W + 2], BF16, kind="ExternalInput")
    # rw[r, i, p*O + o] = rotated filter bank, lhsT layout per tap
    rw = nc.dram_tensor("rw", [R, CIN, 9 * O], BF16, kind="ExternalInput")
    y = nc.dram_tensor("y", [BL, O, H, W], F32, kind="ExternalOutput")

    with TileContext(nc) as tc:
        with (
            tc.tile_pool(name="wrot", bufs=1) as rpool,
            tc.tile_pool(name="xio", bufs=1) as xpool,
            tc.tile_pool(name="accp", bufs=3) as apool,
            tc.tile_pool(name="psum", bufs=1, space="PSUM") as ppool,
        ):
            rotw = [
                rpool.tile([128, 9, O], BF16, name=f"rotw{r}", tag=f"rotw{r}")
                for r in range(R)
            ]

            # PE warm-up: matmuls on a scratch tile seeded by a gpsimd memset
            # (engine-to-engine semaphores are visible in ~1.5us, vs ~4us for
            # DMA-completion semaphores, and gpsimd gates no DMA queue here).
            # Results land in the ps0 bank slot and are overwritten by the
            # first real start=True group.
            dum_lhs = rpool.tile([128, 128], BF16, name="dum_lhs", tag="dum")
            nc.gpsimd.memset(dum_lhs[:, :], 0.0)
            dum_ps = ppool.tile([128, 128], F32, name="dum_ps", tag="ps0")
            for _ in range(WARMUP):
                nc.tensor.matmul(
                    dum_ps[:, :], dum_lhs[:, :], dum_lhs[:, :],
                    start=True, stop=True,
                )

            # x staging: 3 persistent ping-pong buffers, fully overwritten by
            # each block's strip DMA (padding included), so no memsets ever.
            xst2 = [
                xpool.tile([128, RB + 2, W + 2], BF16, name=f"xst{i}", tag=f"xst{i}")
                for i in range(3)
            ]

            def load_x(g, b, blk, chunks=1, cuts=None):
                # DMA the block's padded input rows into the ping-pong
                # staging buffer.  `cuts`/`chunks` split the load so
                # downstream matmuls can start on the first rows before the
                # whole block has landed.
                xst = xst2[g % 3]
                r0 = blk * RB  # padded-row index of the block's top halo row
                nrows = RB + 2
                if cuts is None:
                    cuts = [nrows * k // chunks for k in range(chunks + 1)]
                for k in range(len(cuts) - 1):
                    a, c = cuts[k], cuts[k + 1]
                    nc.sync.dma_start(
                        out=xst[:, a:c, :],
                        in_=xs[b, :, r0 + a : r0 + c, :],
                    )
                return xst

            # Tap emission order: the center tap (ky=1, kx=1) goes first so
            # the start=True matmul covers the full PSUM region; edge taps
            # can then shrink their APs to skip columns/rows that only
            # multiply the zero halo (their contribution is exactly zero).
            TAPS = [4, 0, 1, 2, 3, 5, 6, 7, 8]

            def conv_chunk(xmm, acc, r, blk, store=None, s_groups=1,
                           fine_tail=False):
                top = blk == 0
                bot = blk == NBLK - 1
                pst = [
                    ppool.tile([128, 4, W], F32, name=f"ps{s}", tag=f"ps{s}")
                    for s in range(NS)
                ]

                def emit_max_store(s, rows):
                    # rows: list of (row0, nrows) pieces within the subtile
                    for a, n in rows:
                        lo, hi = 4 * s + a, 4 * s + a + n
                        if r == 0:
                            nc.vector.tensor_copy(
                                acc[:, lo:hi, :], pst[s][:, a : a + n, :]
                            )
                        else:
                            nc.vector.tensor_tensor(
                                acc[:, lo:hi, :],
                                acc[:, lo:hi, :],
                                pst[s][:, a : a + n, :],
                                mybir.AluOpType.max,
                            )
                        if store is not None:
                            b, h0 = store
                            nc.sync.dma_start(
                                out=y[b, :, h0 + lo : h0 + hi, :],
                                in_=acc[:, lo:hi, :],
                            )

                def emit_group(ss):
                    for p in TAPS:
                        ky, kx = divmod(p, 3)
                        lhsT = rotw[r][:, p, :]
                        # out col c <- xmm col c+kx; skip the column that
                        # only reads the zero halo (kx=0: out col 0, kx=2:
                        # out col W-1)
                        if kx == 0:
                            c0, ncol, x0 = 1, W - 1, 1
                        elif kx == 2:
                            c0, ncol, x0 = 0, W - 1, 2
                        else:
                            c0, ncol, x0 = 0, W, 1
                        for s in ss:
                            # skip the output row that only reads the halo
                            # row at the image top/bottom
                            r0, nr = 0, 4
                            if top and s == 0 and ky == 0:
                                r0, nr = 1, 3
                            if bot and s == NS - 1 and ky == 2:
                                nr = 3
                            rhs = xmm[
                                :,
                                4 * s + ky + r0 : 4 * s + ky + r0 + nr,
                                x0 : x0 + ncol,
                            ]
                            nc.tensor.matmul(
                                pst[s][:, r0 : r0 + nr, c0 : c0 + ncol],
                                lhsT, rhs,
                                start=(p == 4), stop=(p == 8),
                            )
                    for s in ss:
                        if fine_tail and s == NS - 1:
                            # drain the last subtile in 2-row pieces so the
                            # final store starts right behind the last matmul
                            emit_max_store(s, [(0, 2), (2, 2)])
                        else:
                            emit_max_store(s, [(0, 4)])

                per = NS // s_groups
                for k in range(s_groups):
                    emit_group(range(k * per, (k + 1) * per))

            # DMA issue order (the sync queue issues serially): rotation 0's
            # ky=0 taps (0-2) and the first 4 x rows go first so the very
            # first matmuls are unblocked right after the queues boot; the
            # remaining taps/rows/rotations follow interleaved by need-time.
            nc.sync.dma_start(
                out=rotw[0][:, 0:3, :].rearrange("i p o -> i (p o)"),
                in_=rw[0, :, 0 : 3 * O],
            )
            xst0 = xst2[0]
            def x0_chunk(a, c):
                nc.sync.dma_start(
                    out=xst0[:, a:c, :],
                    in_=xs[0, :, a:c, :],
                )
            x0_chunk(0, 6)
            nc.sync.dma_start(
                out=rotw[0][:, 3:9, :].rearrange("i p o -> i (p o)"),
                in_=rw[0, :, 3 * O : 9 * O],
            )
            x0_chunk(6, 10)
            x0_chunk(10, 14)
            nc.sync.dma_start(
                out=rotw[1][:, :, :].rearrange("i p o -> i (p o)"),
                in_=rw[1, :, :],
            )
            x0_chunk(14, 24)
            x0_chunk(24, 34)
            for r in range(2, R):
                nc.sync.dma_start(
                    out=rotw[r][:, :, :].rearrange("i p o -> i (p o)"),
                    in_=rw[r, :, :],
                )
            xmm_pre = [xst0]
            xmm_pre.append(load_x(1, 0, 1, chunks=2))
            xmm_pre.append(load_x(2, 0, 2, chunks=2))

            last_g = BL * NBLK - 1
            for g in range(BL * NBLK):
                b, blk = divmod(g, NBLK)
                xmm = xmm_pre[g] if g < 3 else load_x(g, b, blk)
                acc = apool.tile([128, RB, W], F32, name="acc", tag="acc")
                for r in range(R):
                    final = r == R - 1
                    # block 0 rotation 0 runs subtile-major so matmuls start
                    # as soon as the first x rows land; the very last chunk
                    # runs subtile-major so the final stores drain early.
                    # subtile-major everywhere: each PSUM bank's group stops
                    # right before its DVE max, so DVE reads bank s while the
                    # PE writes bank s+1 - never the same bank (p-major
                    # emission made every DVE read overlap PE writes to the
                    # same bank during the next chunk's tap sweeps)
                    sg = 8
                    conv_chunk(
                        xmm, acc, r, blk,
                        store=(b, blk * RB) if final else None,
                        s_groups=sg,
                        fine_tail=(final and g == last_g),
                    )
    nc.finalize()
    return nc


def _get_nc():
    if "v2" not in _NC_CACHE:
        _NC_CACHE["v2"] = _build()
    return _NC_CACHE["v2"]


def kernel(x, weight, rot_alpha):
    global LAST_RESULTS
    x = np.asarray(x, np.float32)
    weight = np.ascontiguousarray(np.asarray(weight, np.float32))
    rot_alpha = np.asarray(rot_alpha, np.float32)

    # Host-side filter rotation: rw[r, i, p, o] = sum_q M[r,p,q] * W[o*R+r, i, q]
    # in f32, then one cast to bf16 (same rounding boundary as the previous
    # on-device DVE mixing, so numerics are unchanged).
    M = _rot_mats(rot_alpha)
    wq = weight.reshape(O, R, CIN, 9).transpose(1, 2, 3, 0)  # (R, I, q, O)
    rot = np.einsum("rpq,riqo->ripo", M, wq)
    rw = np.ascontiguousarray(rot.reshape(R, CIN, 9 * O)).astype(
        ml_dtypes.bfloat16
    )
    xb = np.zeros((B, CIN, H + 2, W + 2), ml_dtypes.bfloat16)
    xb[:, :, 1 : H + 1, 1 : W + 1] = x.astype(ml_dtypes.bfloat16)

    nc = _get_nc()
    in_maps = [
        {"xs": np.ascontiguousarray(xb[c * BL : (c + 1) * BL]), "rw": rw}
        for c in range(NCORES)
    ]
    try:
        res = run_bass_kernel_spmd(nc, in_maps, list(range(NCORES)), trace=_TRACE)
    except Exception:
        # One retry (without tracing): a failed compile or an aborted run can
        # leave a NeuronCore transiently wedged; the next attempt recovers.
        res = run_bass_kernel_spmd(nc, in_maps, list(range(NCORES)), trace=False)
    LAST_RESULTS = res
    return np.concatenate([res.results[c]["y"] for c in range(NCORES)], axis=0)
